# revision 1
# baseline (speedup 1.0000x reference)
"""Fused biased-softmax attention (nn_Attention_55576876810478) on 8 TRN2 NeuronCores.

Tensor-parallel by head (H=8 -> 1 head/core): core h computes head h end to
end -- q/k/v/gate projections, scores with bias_mask+bias_pair, softmax,
P@V, sigmoid gate, and its 32 rows of the output projection -- producing a
partial [B*Q, D] output.  The host sums the 8 partials (the "all-reduce
after linear_o" of the sharding hint, done during unshard) and adds bo.

On-chip layout choices:
  * scores are computed TRANSPOSED, S^T[k, q], so the P@V contraction (over
    k) lands on the partition axis, and bias_mask (a function of k only)
    becomes a per-partition bias folded into the ACT exp instruction.
  * bias_pair arrives host-pre-transposed as bpT[b, kt, k%128, q] (bf16) and
    is accumulated into the scores PSUM with an identity matmul.
  * softmax denominators come for free from the P@V matmul: the stationary
    operand is [V | ones-columns], so row 32+t of the PV accumulator is
    sum_k P[k, q].  Denominators are transposed to [q-partition, 1] columns
    with a tiny K=8 identity matmul, and the divide is applied as a
    per-partition tensor_scalar multiply while evacuating the final matmul.
"""

import math

import ml_dtypes
import numpy as np

B, Q, KL, D, H, C = 4, 1024, 1024, 256, 8, 32
NCORES = 8
BQ = B * Q            # 4096 flattened q positions
BK = B * KL           # 4096 flattened k positions
QT = 512              # q-tile width (free dim of S^T)
KT = 128              # k-tile height (partition dim of S^T)
NQT = BQ // QT        # 8 (b, jq) q-tiles
NKT = KL // KT        # 8 k-tiles per batch
NVG = BK // KT        # 32 global k-tiles (v projection)
NF = BQ // 128        # 32 final output row-tiles

_BF16 = ml_dtypes.bfloat16
_CACHE = {}


def _build_nc():
    import concourse.bass as bass  # noqa: F401
    import concourse.mybir as mybir
    import concourse.tile as tile
    from concourse.bacc import Bacc

    bf16 = mybir.dt.bfloat16
    f32 = mybir.dt.float32
    AF = mybir.ActivationFunctionType
    ALU = mybir.AluOpType

    nc = Bacc(None, target_bir_lowering=False)

    qxT_d = nc.dram_tensor("qxT", [2, 128, BQ], bf16, kind="ExternalInput")
    kvxT_d = nc.dram_tensor("kvxT", [2, 128, BK], bf16, kind="ExternalInput")
    bpT_d = nc.dram_tensor("bpT", [B, NKT, KT, Q], bf16, kind="ExternalInput")
    wqg_d = nc.dram_tensor("wqg", [2, 128, 2 * C], bf16, kind="ExternalInput")
    wk_d = nc.dram_tensor("wk", [2, 128, C], bf16, kind="ExternalInput")
    wv_d = nc.dram_tensor("wv", [2, 128, C], bf16, kind="ExternalInput")
    bg_d = nc.dram_tensor("bg", [2 * C, 1], f32, kind="ExternalInput")
    wo_d = nc.dram_tensor("wo", [C, D], bf16, kind="ExternalInput")
    out_d = nc.dram_tensor("out", [BQ, D], f32, kind="ExternalOutput")

    with tile.TileContext(nc) as tc:
        with (
            tc.tile_pool(name="const", bufs=1) as const,
            tc.tile_pool(name="proj", bufs=1) as proj,
            tc.tile_pool(name="biasp", bufs=17) as biasp,
            tc.tile_pool(name="pp", bufs=8) as pp,
            tc.tile_pool(name="outp", bufs=3) as outp,
        ):
            # ---------------- constants / inputs ----------------
            qxT = const.tile([128, 2, BQ], bf16)
            kvxT = const.tile([128, 2, BK], bf16)
            for dc in range(2):
                nc.sync.dma_start(qxT[:, dc, :], qxT_d[dc])
                nc.sync.dma_start(kvxT[:, dc, :], kvxT_d[dc])
            wqg = const.tile([128, 2, 2 * C], bf16)
            wk = const.tile([128, 2, C], bf16)
            wv = const.tile([128, 2, C], bf16)
            for sb, dr in ((wqg, wqg_d), (wk, wk_d), (wv, wv_d)):
                for dc in range(2):
                    nc.sync.dma_start(sb[:, dc, :], dr[dc])
            bg = const.tile([2 * C, 1], f32)
            nc.sync.dma_start(bg, bg_d[:, :])
            wo = const.tile([C, D], bf16)
            nc.sync.dma_start(wo, wo_d[:, :])

            # persistent intermediates
            qT = proj.tile([C, BQ], bf16)        # [c, b*Q+q]
            qT_r = proj.tile([128, BQ], bf16)    # qT replicated at 4 row groups
            kT_g = proj.tile([128, NVG // 4, KT], bf16)  # group kt%4, block kt//4
            ghi = proj.tile([2 * C, BQ], bf16)   # sigmoid out on partitions 32:64
            gT = proj.tile([33, BQ], bf16)       # sigmoid gate; row 32 = 1.0
            kT = proj.tile([C, BK], bf16)        # [c, b*K+k]
            vones = proj.tile([128, NVG, 33], bf16)  # [k%128, ktile, c|ones]
            odn = proj.tile([33, BQ], bf16)      # gated O^T (rows 0:32) + denom (row 32)
            den4 = proj.tile([128, NF], f32)     # den4[p, 4t+j] = denom(512t+128j+p)
            recip = proj.tile([128, NF], f32)

            nc.vector.memset(vones, 1.0)
            nc.vector.memset(gT[32:33, :], 1.0)

            # ---------------- projections ----------------
            with tc.tile_pool(name="ps_proj", bufs=3, space="PSUM") as ps_pj:
                for j in range(NQT):
                    sl = slice(j * QT, (j + 1) * QT)
                    qg_ps = ps_pj.tile([2 * C, QT], f32, tag="m")
                    for dc in range(2):
                        nc.tensor.matmul(qg_ps, wqg[:, dc, :], qxT[:, dc, sl],
                                         start=dc == 0, stop=dc == 1)
                    nc.vector.tensor_copy(qT[:, sl], qg_ps[0:C, :])
                    # sigmoid(x) = 0.5*tanh(0.5x) + 0.5 -- tanh lives in the
                    # same ACT table set as exp (one table load total)
                    nc.scalar.activation(ghi[C:2 * C, sl], qg_ps[C:2 * C, :],
                                         AF.Tanh, bias=bg[C:2 * C, :],
                                         scale=0.5)
                    nc.vector.tensor_scalar(ghi[C:2 * C, sl],
                                            ghi[C:2 * C, sl], 0.5, 0.5,
                                            op0=ALU.mult, op1=ALU.add)
                    k_ps = ps_pj.tile([C, QT], f32, tag="m")
                    for dc in range(2):
                        nc.tensor.matmul(k_ps, wk[:, dc, :], kvxT[:, dc, sl],
                                         start=dc == 0, stop=dc == 1)
                    nc.vector.tensor_copy(kT[:, sl], k_ps)
                    # prefetch this chunk's share of the kT / qT group layouts
                    for g4 in range(4):
                        nc.gpsimd.dma_start(
                            kT_g[32 * g4:32 * (g4 + 1), j, :],
                            kT[:, (4 * j + g4) * KT:(4 * j + g4 + 1) * KT])
                        nc.gpsimd.dma_start(
                            qT_r[32 * g4:32 * (g4 + 1), sl], qT[:, sl])
                vtt = proj.tile([C, BK], bf16, name="vtt")
                for j in range(NQT):
                    sl = slice(j * QT, (j + 1) * QT)
                    v_ps = ps_pj.tile([C, QT], f32, tag="m")
                    for dc in range(2):
                        nc.tensor.matmul(v_ps, wv[:, dc, :], kvxT[:, dc, sl],
                                         start=dc == 0, stop=dc == 1)
                    nc.vector.tensor_copy(vtt[:, sl], v_ps)
                # 32x32 block transpose: vtb[a, 32*fb+b] = v[k=32*fb+a, c=b]
                vtb = proj.tile([C, BK], bf16, name="vtb")
                nc.vector.transpose(vtb, vtt)
                # remap to vones[k%128, ktile, c] (4 DMAs, one per k%128
                # quarter: dst partitions 32qq..32qq+32 <- src partitions 0:32)
                vtb_v = vtb.rearrange("a (g rest) -> a g rest", rest=4 * C)
                for qq in range(4):
                    nc.gpsimd.dma_start(
                        vones[32 * qq:32 * (qq + 1), :, 0:C],
                        vtb_v[:, :, 32 * qq:32 * qq + C])
            # relocate gate rows 32:64 -> 0:32 (SBUF->SBUF DMA partition remap)
            nc.gpsimd.dma_start(gT[0:C, :], ghi[C:2 * C, :])

            # ---------------- attention ----------------
            with (
                tc.tile_pool(name="ps_s", bufs=5, space="PSUM") as ps_s,
                tc.tile_pool(name="ps_pv", bufs=2, space="PSUM") as ps_pv,
                tc.tile_pool(name="ps_f", bufs=1, space="PSUM") as ps_f,
            ):
                bp_tiles = {}

                def bias_prefetch(bb):
                    for kt in range(NKT):
                        bp = biasp.tile([128, Q], bf16, tag="bias",
                                        name=f"bp_{bb}_{kt}")
                        nc.sync.dma_start(bp, bpT_d[bb, kt])
                        bp_tiles[(bb, kt)] = bp

                bias_prefetch(0)
                for b in range(B):
                    if b + 1 < B:
                        bias_prefetch(b + 1)
                    pv = [ps_pv.tile([33, QT], f32, tag="pv", name=f"pv_{b}_{i}")
                          for i in range(2)]
                    for pk in range(2):
                        bps = [bp_tiles.pop((b, 4 * pk + g4)) for g4 in range(4)]
                        for jq in range(2):
                            qsl = slice(b * Q + jq * QT, b * Q + (jq + 1) * QT)
                            sb = []
                            # 4x row-packed score matmuls (K=32 each)
                            for g4 in range(4):
                                s = ps_s.tile([128, QT], f32, tag="s",
                                              name=f"s_{b}_{pk}_{jq}_{g4}")
                                nc.tensor.matmul(
                                    s, kT_g[32 * g4:32 * (g4 + 1), 2 * b + pk, :],
                                    qT_r[32 * g4:32 * (g4 + 1), qsl],
                                    start=True, stop=True,
                                    tile_position=(32 * g4, 0))
                                sb.append(s)
                            for g4 in range(4):
                                kt = 4 * pk + g4
                                gk = b * NKT + kt
                                praw = pp.tile([128, QT], bf16, tag="praw")
                                nc.scalar.activation(praw, sb[g4], AF.Exp)
                                p = pp.tile([128, QT], bf16, tag="p")
                                # P = exp(S) * exp(bias_pair + bias_mask)
                                # (bf16 2x-mode DVE multiply)
                                nc.vector.tensor_mul(
                                    p, praw,
                                    bps[g4][:, jq * QT:(jq + 1) * QT])
                                nc.tensor.matmul(
                                    pv[jq][0:33, :], vones[:, gk, :], p,
                                    start=kt == 0, stop=kt == NKT - 1)
                    out_r = out_d[:, :].rearrange("(t p j) d -> t j p d",
                                                  p=128, j=4)
                    for jq in range(2):
                        t = 2 * b + jq
                        qsl = slice(b * Q + jq * QT, b * Q + (jq + 1) * QT)
                        # odn = (pv * 1.0) * [gate; 1]  (fused evict + gate
                        # mult; row 32 passes the denominator through)
                        nc.vector.scalar_tensor_tensor(
                            odn[:, qsl], pv[jq][0:33, :], 1.0,
                            gT[:, qsl], op0=ALU.mult, op1=ALU.mult)
                        # denominators of this q-tile -> per-partition
                        # columns: den4[p, 4t+j] = denom(q = 512t + 4p + j)
                        # (the DMA walks dst (p, j) in order, consuming the
                        # source row linearly)
                        nc.gpsimd.dma_start(den4[:, 4 * t:4 * t + 4],
                                            odn[32:33, qsl])
                        nc.vector.reciprocal(recip[:, 4 * t:4 * t + 4],
                                             den4[:, 4 * t:4 * t + 4])
                        # output projection: tile (t, j) covers the stride-4
                        # q-set {512t + 4p + j}
                        og_t = odn[0:C, qsl].rearrange("c (p j) -> c j p", j=4)
                        for j4 in range(4):
                            f = 4 * t + j4
                            fo = ps_f.tile([128, D], f32, tag="f",
                                           name=f"fo_{f}")
                            nc.tensor.matmul(fo, og_t[:, j4, :], wo,
                                             start=True, stop=True)
                            ot = outp.tile([128, D], f32, tag="ot",
                                           name=f"ot_{f}")
                            if f % 2 == 0:
                                nc.vector.tensor_scalar_mul(ot, fo,
                                                            recip[:, f:f + 1])
                            else:
                                nc.scalar.activation(ot, fo, AF.Copy,
                                                     scale=recip[:, f:f + 1])
                            nc.sync.dma_start(out_r[t, j4], ot)

    nc.finalize()
    return nc


def _get_nc():
    if "nc" not in _CACHE:
        _CACHE["nc"] = _build_nc()
    return _CACHE["nc"]


def _prep(inputs):
    q_x = np.asarray(inputs["q_x"], np.float32)
    kv_x = np.asarray(inputs["kv_x"], np.float32)
    bias_mask = np.asarray(inputs["bias_mask"], np.float32)
    bias_pair = np.asarray(inputs["bias_pair"], np.float32)
    wq = np.asarray(inputs["wq"], np.float32)
    wk = np.asarray(inputs["wk"], np.float32)
    wv = np.asarray(inputs["wv"], np.float32)
    wg = np.asarray(inputs["wg"], np.float32)
    bg = np.asarray(inputs["bg"], np.float32)
    wo = np.asarray(inputs["wo"], np.float32)

    qxT = np.ascontiguousarray(q_x.reshape(BQ, D).T).astype(_BF16).reshape(2, 128, BQ)
    kvxT = np.ascontiguousarray(kv_x.reshape(BK, D).T).astype(_BF16).reshape(2, 128, BK)
    bmk = bias_mask.reshape(B, KL)
    sc = 1.0 / math.sqrt(C)

    in_maps = []
    for h in range(NCORES):
        csl = slice(h * C, (h + 1) * C)
        bpT = np.exp(bias_pair[:, h].transpose(0, 2, 1)
                     + bmk[:, :, None]).astype(_BF16)
        bpT = bpT.reshape(B, NKT, KT, Q)
        in_maps.append({
            "qxT": qxT,
            "kvxT": kvxT,
            "bpT": bpT,
            "wqg": np.ascontiguousarray(
                np.concatenate([wq[:, csl] * sc, wg[:, csl]], axis=1)
            ).astype(_BF16).reshape(2, 128, 2 * C),
            "wk": np.ascontiguousarray(wk[:, csl]).astype(_BF16).reshape(2, 128, C),
            "wv": np.ascontiguousarray(wv[:, csl]).astype(_BF16).reshape(2, 128, C),
            "bg": np.concatenate(
                [np.zeros(C, np.float32), 0.5 * bg[csl].astype(np.float32)]
            ).reshape(2 * C, 1),
            "wo": np.ascontiguousarray(wo[csl, :]).astype(_BF16),
        })
    return in_maps


def _run(inputs, trace=False, **kw):
    from concourse.bass_utils import run_bass_kernel_spmd

    in_maps = _prep(inputs)
    nc = _get_nc()
    r = run_bass_kernel_spmd(nc, in_maps, core_ids=list(range(NCORES)),
                             trace=trace, **kw)
    bo = np.asarray(inputs["bo"], np.float32)
    total = np.zeros((BQ, D), np.float32)
    for i in range(NCORES):
        total += r.results[i]["out"].reshape(BQ, D).astype(np.float32)
    total += bo
    return total.reshape(B, Q, D).astype(np.float32), r


def kernel(**inputs):
    out, _ = _run(inputs, trace=False)
    return out



# revision 13
# speedup vs baseline: 1.2009x; 1.2009x over previous
"""Fused biased-softmax attention (nn_Attention_55576876810478) on 8 TRN2 NeuronCores.

Sharding: 2-D (batch x head-group).  Core c = (b, hg) with b = c//2, hg = c%2
owns batch b and heads 4*hg .. 4*hg+3 (4 heads x 32 ch = 128 columns of every
projection).  Each core computes its batch's q/k/v/gate projections for its 4
heads, the biased softmax attention, the sigmoid gate, the per-head softmax
normalization, and its 128 rows of the output projection -- producing a
partial [Q, D] output per core.  The host sums each batch's two partials and
adds bo.

Key on-chip structure (per core):
  * projections: stationaries [256, 128] -> psum [128(h,c), 512-token chunks];
    the (h, c) row layout IS the 4-row-band layout the packed score matmuls
    need, so there are no relayout DMAs for q/k at all.
  * v is projected with kvx^T chunks as the *stationary* (out = v[k, (h,c)]),
    which lands v directly in the [k%128, ktile, c] layout the PV matmul
    wants -- no transposes.
  * scores are computed transposed, S^T[k, q], 4 heads packed in the 4 PE
    row-bands (concurrent matmuls).  bias_pair + bias_mask arrive raw (bf16)
    and are accumulated into the score PSUM with an identity matmul before a
    single exp -- no separate bias multiply pass.
  * exp runs on [128, 1024] 2-bank psum tiles (amortizes ACT overhead).
  * PV uses a [v | ones] stationary (row 32 of each accumulator = softmax
    denominator) with two heads col-packed per PSUM bank (bases 0 / 64).
  * per-head 1/den is broadcast to the head's 32 channel rows with gpsimd
    partition_broadcast and folded into the gated O^T before the final
    projection, whose 4 per-head matmuls accumulate in one PSUM bank.
"""

import math

import ml_dtypes
import numpy as np

B, Q, KL, D, H, C = 4, 1024, 1024, 256, 8, 32
NCORES = 8
NKT = KL // 128        # 8 k-tiles
NCH = Q // 512         # 2 512-token chunks

_BF16 = ml_dtypes.bfloat16
_CACHE = {}


def _build_nc():
    import concourse.bass as bass  # noqa: F401
    import concourse.mybir as mybir
    import concourse.tile as tile
    from concourse.bacc import Bacc

    bf16 = mybir.dt.bfloat16
    f32 = mybir.dt.float32
    AF = mybir.ActivationFunctionType
    ALU = mybir.AluOpType

    nc = Bacc(None, target_bir_lowering=False)

    qxT_d = nc.dram_tensor("qxT", [2, 128, Q], bf16, kind="ExternalInput")
    kvxT_d = nc.dram_tensor("kvxT", [2, 128, KL], bf16, kind="ExternalInput")
    wq4_d = nc.dram_tensor("wq4", [2, 128, 128], bf16, kind="ExternalInput")
    wg4_d = nc.dram_tensor("wg4", [2, 128, 128], bf16, kind="ExternalInput")
    wk4_d = nc.dram_tensor("wk4", [2, 128, 128], bf16, kind="ExternalInput")
    wv4_d = nc.dram_tensor("wv4", [2, 128, 128], bf16, kind="ExternalInput")
    bg4_d = nc.dram_tensor("bg4", [128, 1], f32, kind="ExternalInput")
    biasT_d = nc.dram_tensor("biasT", [4, 4, 128, 2048], bf16,
                             kind="ExternalInput")
    ident_d = nc.dram_tensor("ident", [128, 128], bf16, kind="ExternalInput")
    wo4_d = nc.dram_tensor("wo4", [128, D], bf16, kind="ExternalInput")
    out_d = nc.dram_tensor("out", [Q, D], f32, kind="ExternalOutput")
    out_r = out_d[:, :].rearrange("(t p) d -> t p d", p=128)

    with tile.TileContext(nc) as tc:
        with (
            tc.tile_pool(name="const", bufs=1) as const,
            tc.tile_pool(name="biasp", bufs=1) as biasp,
            tc.tile_pool(name="proj", bufs=1) as proj,
            tc.tile_pool(name="pp", bufs=4) as pp,
            tc.tile_pool(name="dnp", bufs=2) as dnp,
            tc.tile_pool(name="outp", bufs=3) as outp,
        ):
            # ---------------- input DMAs ----------------
            qxT = const.tile([128, 2, Q], bf16)
            kvxT = const.tile([128, 2, KL], bf16)
            wq4 = const.tile([128, 2, 128], bf16)
            wg4 = const.tile([128, 2, 128], bf16)
            wk4 = const.tile([128, 2, 128], bf16)
            wv4 = const.tile([128, 2, 128], bf16)
            for sb, dr in ((qxT, qxT_d), (kvxT, kvxT_d), (wq4, wq4_d),
                           (wg4, wg4_d), (wk4, wk4_d), (wv4, wv4_d)):
                for dc in range(2):
                    nc.sync.dma_start(sb[:, dc, :], dr[dc])
            bg4 = const.tile([128, 1], f32)
            nc.sync.dma_start(bg4, bg4_d[:, :])
            ident = const.tile([128, 128], bf16)
            nc.sync.dma_start(ident, ident_d[:, :])
            wo4 = const.tile([128, D], bf16)
            nc.sync.dma_start(wo4, wo4_d[:, :])
            # bias tiles: bp[j][ktpair] = [128, (kt2, q)]; issued in
            # consumption order (ktpair-major)
            bp = [[None] * 4 for _ in range(4)]
            for ktp in range(4):
                for j in range(4):
                    t = biasp.tile([128, 2048], bf16, tag=f"bp{j}_{ktp}",
                                   name=f"bp{j}_{ktp}")
                    nc.sync.dma_start(t, biasT_d[j, ktp])
                    bp[j][ktp] = t

            # ---------------- persistent intermediates ----------------
            qTb = proj.tile([128, Q], bf16)      # q^T, rows (h, c)
            kTb = proj.tile([128, KL], bf16)     # k^T, rows (h, c)
            gT = proj.tile([128, Q], bf16)       # sigmoid gate, rows (h, c)
            gT33 = proj.tile([128, 4, Q], bf16)  # per-head gate + ones row 32
            vones = proj.tile([128, 4, NKT, 33], bf16)  # [k%128, h, kt, c|1]
            odnA = proj.tile([128, Q], bf16)     # gated O^T + den (pair h0,h1)
            odnB = proj.tile([128, Q], bf16)     # pair h2,h3
            ogsS = proj.tile([128, Q], bf16)     # normalized gated O^T, rows (h, c)

            nc.vector.memset(vones, 1.0)
            nc.vector.memset(gT33[32:33, :, :], 1.0)
            zst = proj.tile([128, 33], bf16)
            nc.vector.memset(zst, 0.0)

            # ---------------- projections ----------------
            with tc.tile_pool(name="ps_pj", bufs=3, space="PSUM") as ps_pj:
                for ch in range(NCH):
                    sl = slice(ch * 512, (ch + 1) * 512)
                    q_ps = ps_pj.tile([128, 512], f32, tag="pj")
                    for dc in range(2):
                        nc.tensor.matmul(q_ps, wq4[:, dc, :], qxT[:, dc, sl],
                                         start=dc == 0, stop=dc == 1)
                    nc.vector.tensor_copy(qTb[:, sl], q_ps)
                    k_ps = ps_pj.tile([128, 512], f32, tag="pj")
                    for dc in range(2):
                        nc.tensor.matmul(k_ps, wk4[:, dc, :], kvxT[:, dc, sl],
                                         start=dc == 0, stop=dc == 1)
                    nc.vector.tensor_copy(kTb[:, sl], k_ps)
                    g_ps = ps_pj.tile([128, 512], f32, tag="pj")
                    for dc in range(2):
                        nc.tensor.matmul(g_ps, wg4[:, dc, :], qxT[:, dc, sl],
                                         start=dc == 0, stop=dc == 1)
                    # sigmoid(x) = 0.5*tanh(0.5x) + 0.5 (tanh shares the exp
                    # ACT table set)
                    nc.scalar.activation(gT[:, sl], g_ps, AF.Tanh,
                                         bias=bg4, scale=0.5)
                    nc.vector.tensor_scalar(gT[:, sl], gT[:, sl], 0.5, 0.5,
                                            op0=ALU.mult, op1=ALU.add)
                # v projected with kvx^T as stationary: v_ps[k, (h, c)]
                for kt in range(NKT):
                    ksl = slice(kt * 128, (kt + 1) * 128)
                    v_ps = ps_pj.tile([128, 128], f32, tag="vps")
                    for dc in range(2):
                        nc.tensor.matmul(v_ps, kvxT[:, dc, ksl], wv4[:, dc, :],
                                         start=dc == 0, stop=dc == 1)
                    nc.vector.tensor_copy(vones[:, :, kt, 0:C], v_ps)
            # per-head gate rows -> gT33 (ones row 32 already set)
            for j in range(4):
                nc.sync.dma_start(gT33[0:C, j, :], gT[32 * j:32 * (j + 1), :])

            # ---------------- attention ----------------
            with (
                tc.tile_pool(name="ps_s", bufs=3, space="PSUM") as ps_s,
                tc.tile_pool(name="ps_pv", bufs=2, space="PSUM") as ps_pv,
            ):
                for jq in range(2):
                    qsl = slice(jq * 512, (jq + 1) * 512)
                    pvA = ps_pv.tile([128, 512], f32, tag="pv",
                                     name=f"pvA_{jq}")
                    pvB = ps_pv.tile([128, 512], f32, tag="pv",
                                     name=f"pvB_{jq}")
                    # zero-init the upper (base-64) col-tile region: its
                    # accumulating matmuls use start=False (the lower tile's
                    # start clears the whole bank's has_written bits on HW)
                    for pv in (pvA, pvB):
                        nc.tensor.matmul(pv[64:97, :], zst, qTb[:, qsl],
                                         start=True, stop=False,
                                         tile_position=(0, 64),
                                         skip_group_check=True)

                    def emit_pv(kt, prawA, prawB):
                        for pv, praw, jlo in ((pvA, prawA, 0), (pvB, prawB, 2)):
                            nc.tensor.matmul(
                                pv[0:33, :], vones[:, jlo, kt, :],
                                praw[:, 0:512],
                                start=kt == 0, stop=kt == NKT - 1,
                                tile_position=(0, 0))
                        for pv, praw, jlo in ((pvA, prawA, 0), (pvB, prawB, 2)):
                            nc.tensor.matmul(
                                pv[64:97, :], vones[:, jlo + 1, kt, :],
                                praw[:, 512:1024],
                                start=False, stop=kt == NKT - 1,
                                tile_position=(0, 64),
                                skip_group_check=True)

                    prev = None
                    for kt in range(NKT):
                        ktp, kt2 = kt // 2, kt % 2
                        ksl = slice(kt * 128, (kt + 1) * 128)
                        sA = ps_s.tile([128, 1024], f32, tag="s",
                                       name=f"sA_{jq}_{kt}")
                        sB = ps_s.tile([128, 1024], f32, tag="s",
                                       name=f"sB_{jq}_{kt}")
                        # 4 packed score matmuls (row band = head)
                        for j in range(4):
                            s_t = sA if j < 2 else sB
                            half = slice((j % 2) * 512, (j % 2) * 512 + 512)
                            nc.tensor.matmul(
                                s_t[:, half],
                                kTb[32 * j:32 * (j + 1), ksl],
                                qTb[32 * j:32 * (j + 1), qsl],
                                start=True, stop=False,
                                tile_position=(32 * j, 0))
                        # bias accumulate via identity matmul
                        for j in range(4):
                            s_t = sA if j < 2 else sB
                            half = slice((j % 2) * 512, (j % 2) * 512 + 512)
                            bsl = slice(kt2 * 1024 + jq * 512,
                                        kt2 * 1024 + jq * 512 + 512)
                            nc.tensor.matmul(
                                s_t[:, half], ident, bp[j][ktp][:, bsl],
                                start=False, stop=True)
                        prawA = pp.tile([128, 1024], bf16, tag="praw",
                                        name=f"prawA_{jq}_{kt}")
                        nc.scalar.activation(prawA, sA, AF.Exp)
                        prawB = pp.tile([128, 1024], bf16, tag="praw",
                                        name=f"prawB_{jq}_{kt}")
                        nc.scalar.activation(prawB, sB, AF.Exp)
                        if prev is not None:
                            emit_pv(*prev)
                        prev = (kt, prawA, prawB)
                    emit_pv(*prev)

                    # gate + extract denominators, normalize, project out
                    # upper-half STT first: it waits on the bank's LAST
                    # matmul (h1/h3 kt=7), so the later lower-half read can't
                    # collide with an in-flight PE write to the same bank
                    for pv, odn, jlo in ((pvA, odnA, 0), (pvB, odnB, 2)):
                        nc.vector.scalar_tensor_tensor(
                            odn[64:97, qsl], pv[64:97, :], 1.0,
                            gT33[0:33, jlo + 1, qsl],
                            op0=ALU.mult, op1=ALU.mult)
                        nc.vector.scalar_tensor_tensor(
                            odn[0:33, qsl], pv[0:33, :], 1.0,
                            gT33[0:33, jlo, qsl],
                            op0=ALU.mult, op1=ALU.mult)
                    for odn, jlo, tagn in ((odnA, 0, "a"), (odnB, 2, "b")):
                        for ih, base in ((0, 0), (1, 64)):
                            rbh = dnp.tile([1, 512], bf16, tag="rbh",
                                           name=f"rbh{tagn}_{jq}_{base}")
                            with nc.allow_low_precision(
                                    "bf16 1/den: softmax scale, 0.4% ok"):
                                nc.vector.reciprocal(
                                    rbh, odn[base + C:base + C + 1, qsl])
                            # partition_broadcast requires out base 0: fill
                            # all 128 rows, consume the slice we need
                            rbig = dnp.tile([128, 512], bf16, tag="rbig",
                                            name=f"rbig{tagn}_{jq}_{base}")
                            nc.gpsimd.partition_broadcast(rbig, rbh[0:1, :],
                                                          channels=128)
                            # stack normalized head rows at 32*head for a
                            # single full-contraction output matmul
                            j = jlo + ih
                            nc.vector.tensor_mul(ogsS[32 * j:32 * j + C, qsl],
                                                 odn[base:base + C, qsl],
                                                 rbig[base:base + C, :])
                    for qt in range(4):
                        qq = slice(jq * 512 + qt * 128,
                                   jq * 512 + qt * 128 + 128)
                        fo = ps_pv.tile([128, 512], f32, tag="pv",
                                        name=f"fo_{jq}_{qt}")
                        nc.tensor.matmul(fo[:, 0:D], ogsS[:, qq], wo4,
                                         start=True, stop=True)
                        ot = outp.tile([128, D], f32, tag="ot",
                                       name=f"ot_{jq}_{qt}")
                        nc.vector.tensor_copy(ot, fo[:, 0:D])
                        nc.sync.dma_start(out_r[jq * 4 + qt], ot)

    nc.finalize()
    return nc


def _get_nc():
    if "nc" not in _CACHE:
        _CACHE["nc"] = _build_nc()
    return _CACHE["nc"]


def _prep(inputs):
    q_x = np.asarray(inputs["q_x"], np.float32)
    kv_x = np.asarray(inputs["kv_x"], np.float32)
    bias_mask = np.asarray(inputs["bias_mask"], np.float32)
    bias_pair = np.asarray(inputs["bias_pair"], np.float32)
    wq = np.asarray(inputs["wq"], np.float32)
    wk = np.asarray(inputs["wk"], np.float32)
    wv = np.asarray(inputs["wv"], np.float32)
    wg = np.asarray(inputs["wg"], np.float32)
    bg = np.asarray(inputs["bg"], np.float32)
    wo = np.asarray(inputs["wo"], np.float32)

    sc = 1.0 / math.sqrt(C)
    ident = np.eye(128, dtype=_BF16)
    bmk = bias_mask.reshape(B, KL)

    in_maps = []
    for core in range(NCORES):
        b, hg = core // 2, core % 2
        hsl = slice(hg * 128, (hg + 1) * 128)
        qxT = np.ascontiguousarray(q_x[b].T).astype(_BF16).reshape(2, 128, Q)
        kvxT = np.ascontiguousarray(kv_x[b].T).astype(_BF16).reshape(2, 128, KL)
        # bias: [4h, K, Q] -> [4h, ktpair, p, kt2, q] -> [4, 4, 128, 2048]
        bT = (bias_pair[b, 4 * hg:4 * hg + 4].transpose(0, 2, 1)
              + bmk[b][None, :, None])
        bT = bT.reshape(4, 4, 2, 128, Q).transpose(0, 1, 3, 2, 4)
        bT = np.ascontiguousarray(bT).astype(_BF16).reshape(4, 4, 128, 2048)

        in_maps.append({
            "qxT": qxT,
            "kvxT": kvxT,
            "wq4": np.ascontiguousarray(wq[:, hsl] * sc).astype(_BF16).reshape(2, 128, 128),
            "wg4": np.ascontiguousarray(wg[:, hsl]).astype(_BF16).reshape(2, 128, 128),
            "wk4": np.ascontiguousarray(wk[:, hsl]).astype(_BF16).reshape(2, 128, 128),
            "wv4": np.ascontiguousarray(wv[:, hsl]).astype(_BF16).reshape(2, 128, 128),
            "bg4": (0.5 * bg[hsl]).astype(np.float32).reshape(128, 1),
            "biasT": bT,
            "ident": ident,
            "wo4": np.ascontiguousarray(wo[hsl]).astype(_BF16),
        })
    return in_maps


def _run(inputs, trace=False, **kw):
    from concourse.bass_utils import run_bass_kernel_spmd

    in_maps = _prep(inputs)
    nc = _get_nc()
    r = run_bass_kernel_spmd(nc, in_maps, core_ids=list(range(NCORES)),
                             trace=trace, **kw)
    bo = np.asarray(inputs["bo"], np.float32)
    out = np.zeros((B, Q, D), np.float32)
    for b in range(B):
        out[b] = (r.results[2 * b]["out"].astype(np.float32)
                  + r.results[2 * b + 1]["out"].astype(np.float32) + bo)
    return out, r


def kernel(**inputs):
    out, _ = _run(inputs, trace=False)
    return out


# revision 18
# speedup vs baseline: 1.3232x; 1.1018x over previous
"""Fused biased-softmax attention (nn_Attention_55576876810478) on 8 TRN2 NeuronCores.

Sharding: 2-D (batch x head-group).  Core c = (b, hg) with b = c//2, hg = c%2
owns batch b and heads 4*hg .. 4*hg+3 (4 heads x 32 ch = 128 columns of every
projection).  Each core computes its batch's q/k/v/gate projections for its 4
heads, the biased softmax attention, the sigmoid gate, the per-head softmax
normalization, and its 128 rows of the output projection -- producing a
partial [Q, D] output per core.  The host sums each batch's two partials and
adds bo.

Key on-chip structure (per core):
  * projections: stationaries [256, 128] -> psum [128(h,c), 512-token chunks];
    the (h, c) row layout IS the 4-row-band layout the packed score matmuls
    need, so there are no relayout DMAs for q/k at all.
  * v is projected with kvx^T chunks as the *stationary* (out = v[k, (h,c)]),
    which lands v directly in the [k%128, ktile, c] layout the PV matmul
    wants -- no transposes.
  * scores are computed transposed, S^T[k, q], 4 heads packed in the 4 PE
    row-bands (concurrent matmuls).  bias_pair + bias_mask arrive raw (bf16)
    and are accumulated into the score PSUM with an identity matmul before a
    single exp -- no separate bias multiply pass.
  * exp runs on [128, 1024] 2-bank psum tiles (amortizes ACT overhead).
  * PV uses a [v | ones] stationary (row 32 of each accumulator = softmax
    denominator) with two heads col-packed per PSUM bank (bases 0 / 64).
  * per-head 1/den is broadcast to the head's 32 channel rows with gpsimd
    partition_broadcast and folded into the gated O^T before the final
    projection, whose 4 per-head matmuls accumulate in one PSUM bank.
"""

import math

import ml_dtypes
import numpy as np

B, Q, KL, D, H, C = 4, 1024, 1024, 256, 8, 32
NCORES = 8
NKT = KL // 128        # 8 k-tiles
NCH = Q // 512         # 2 512-token chunks

_BF16 = ml_dtypes.bfloat16
_CACHE = {}


def _build_nc():
    import concourse.bass as bass  # noqa: F401
    import concourse.mybir as mybir
    import concourse.tile as tile
    from concourse.bacc import Bacc

    bf16 = mybir.dt.bfloat16
    f32 = mybir.dt.float32
    AF = mybir.ActivationFunctionType
    ALU = mybir.AluOpType

    nc = Bacc(None, target_bir_lowering=False)

    qxT_d = nc.dram_tensor("qxT", [2, 128, Q], bf16, kind="ExternalInput")
    kvxT_d = nc.dram_tensor("kvxT", [2, 128, KL], bf16, kind="ExternalInput")
    wq4_d = nc.dram_tensor("wq4", [2, 128, 128], bf16, kind="ExternalInput")
    wg4_d = nc.dram_tensor("wg4", [2, 128, 128], bf16, kind="ExternalInput")
    wk4_d = nc.dram_tensor("wk4", [2, 128, 128], bf16, kind="ExternalInput")
    wv4_d = nc.dram_tensor("wv4", [2, 128, 128], bf16, kind="ExternalInput")
    bg4_d = nc.dram_tensor("bg4", [128, 1], f32, kind="ExternalInput")
    biasT_d = nc.dram_tensor("biasT", [4, 4, 128, 2048], bf16,
                             kind="ExternalInput")
    ident_d = nc.dram_tensor("ident", [128, 128], bf16, kind="ExternalInput")
    woh_d = nc.dram_tensor("woh", [32, 4 * D], bf16, kind="ExternalInput")
    out_d = nc.dram_tensor("out", [Q, D], f32, kind="ExternalOutput")
    out_r = out_d[:, :].rearrange("(t p j) d -> t j p d", p=128, j=4)

    with tile.TileContext(nc) as tc:
        with (
            tc.tile_pool(name="const", bufs=1) as const,
            tc.tile_pool(name="biasp", bufs=1) as biasp,
            tc.tile_pool(name="proj", bufs=1) as proj,
            tc.tile_pool(name="pp", bufs=4) as pp,
            tc.tile_pool(name="dnp", bufs=2) as dnp,
            tc.tile_pool(name="outp", bufs=3) as outp,
        ):
            # ---------------- input DMAs ----------------
            qxT = const.tile([128, 2, Q], bf16)
            kvxT = const.tile([128, 2, KL], bf16)
            wq4 = const.tile([128, 2, 128], bf16)
            wg4 = const.tile([128, 2, 128], bf16)
            wk4 = const.tile([128, 2, 128], bf16)
            wv4 = const.tile([128, 2, 128], bf16)
            for sb, dr in ((qxT, qxT_d), (wq4, wq4_d), (kvxT, kvxT_d),
                           (wk4, wk4_d), (wg4, wg4_d), (wv4, wv4_d)):
                for dc in range(2):
                    nc.sync.dma_start(sb[:, dc, :], dr[dc])
            bg4 = const.tile([128, 1], f32)
            nc.sync.dma_start(bg4, bg4_d[:, :])
            ident = const.tile([128, 128], bf16)
            nc.sync.dma_start(ident, ident_d[:, :])
            woh = const.tile([32, 4, D], bf16)
            nc.sync.dma_start(woh, woh_d[:, :])
            # bias tiles: bp[j][ktpair] = [128, (kt2, q)]; issued in
            # consumption order (ktpair-major)
            bp = [[None] * 4 for _ in range(4)]
            for ktp in range(4):
                for j in range(4):
                    t = biasp.tile([128, 2048], bf16, tag=f"bp{j}_{ktp}",
                                   name=f"bp{j}_{ktp}")
                    nc.gpsimd.dma_start(t, biasT_d[j, ktp])
                    bp[j][ktp] = t

            # ---------------- persistent intermediates ----------------
            qTb = proj.tile([128, Q], bf16)      # q^T, rows (h, c)
            kTb = proj.tile([128, KL], bf16)     # k^T, rows (h, c)
            gT = proj.tile([128, Q], bf16)       # sigmoid gate, rows (h, c)
            gT33 = proj.tile([128, 4, Q], bf16)  # per-head gate + ones row 32
            vones = proj.tile([128, 4, NKT, 33], bf16)  # [k%128, h, kt, c|1]
            # per-head gated O^T + den row 32, all at base partition 0
            odn = [proj.tile([33, Q], bf16, name=f"odn{j}") for j in range(4)]
            den4 = proj.tile([128, 32], bf16)    # den4[p, 8h+4jq+j4]
            recip4 = proj.tile([128, 32], f32)

            nc.vector.memset(vones, 1.0)
            nc.vector.memset(gT33[32:33, :, :], 1.0)
            zst = proj.tile([128, 33], bf16)
            nc.vector.memset(zst, 0.0)

            # ---------------- projections ----------------
            with tc.tile_pool(name="ps_pj", bufs=3, space="PSUM") as ps_pj:
                for ch in range(NCH):
                    sl = slice(ch * 512, (ch + 1) * 512)
                    q_ps = ps_pj.tile([128, 512], f32, tag="pj")
                    for dc in range(2):
                        nc.tensor.matmul(q_ps, wq4[:, dc, :], qxT[:, dc, sl],
                                         start=dc == 0, stop=dc == 1)
                    nc.vector.tensor_copy(qTb[:, sl], q_ps)
                    k_ps = ps_pj.tile([128, 512], f32, tag="pj")
                    for dc in range(2):
                        nc.tensor.matmul(k_ps, wk4[:, dc, :], kvxT[:, dc, sl],
                                         start=dc == 0, stop=dc == 1)
                    nc.vector.tensor_copy(kTb[:, sl], k_ps)
                    g_ps = ps_pj.tile([128, 512], f32, tag="pj")
                    for dc in range(2):
                        nc.tensor.matmul(g_ps, wg4[:, dc, :], qxT[:, dc, sl],
                                         start=dc == 0, stop=dc == 1)
                    # sigmoid(x) = 0.5*tanh(0.5x) + 0.5 (tanh shares the exp
                    # ACT table set)
                    nc.scalar.activation(gT[:, sl], g_ps, AF.Tanh,
                                         bias=bg4, scale=0.5)
                    nc.vector.tensor_scalar(gT[:, sl], gT[:, sl], 0.5, 0.5,
                                            op0=ALU.mult, op1=ALU.add)
                # v projected with kvx^T as stationary: v_ps[k, (h, c)]
                for kt in range(NKT):
                    ksl = slice(kt * 128, (kt + 1) * 128)
                    v_ps = ps_pj.tile([128, 128], f32, tag="vps")
                    for dc in range(2):
                        nc.tensor.matmul(v_ps, kvxT[:, dc, ksl], wv4[:, dc, :],
                                         start=dc == 0, stop=dc == 1)
                    nc.vector.tensor_copy(vones[:, :, kt, 0:C], v_ps)
            # per-head gate rows -> gT33 (ones row 32 already set)
            for j in range(4):
                nc.sync.dma_start(gT33[0:C, j, :], gT[32 * j:32 * (j + 1), :])

            # ---------------- attention ----------------
            with (
                tc.tile_pool(name="ps_s", bufs=3, space="PSUM") as ps_s,
                tc.tile_pool(name="ps_pv", bufs=2, space="PSUM") as ps_pv,
            ):
                for jq in range(2):
                    qsl = slice(jq * 512, (jq + 1) * 512)
                    pvA = ps_pv.tile([128, 512], f32, tag="pv",
                                     name=f"pvA_{jq}")
                    pvB = ps_pv.tile([128, 512], f32, tag="pv",
                                     name=f"pvB_{jq}")
                    # zero-init the upper (base-64) col-tile region: its
                    # accumulating matmuls use start=False (the lower tile's
                    # start clears the whole bank's has_written bits on HW)
                    for pv in (pvA, pvB):
                        nc.tensor.matmul(pv[64:97, :], zst, qTb[:, qsl],
                                         start=True, stop=False,
                                         tile_position=(0, 64),
                                         skip_group_check=True)

                    def emit_pv(kt, prawA, prawB):
                        for pv, praw, jlo in ((pvA, prawA, 0), (pvB, prawB, 2)):
                            nc.tensor.matmul(
                                pv[0:33, :], vones[:, jlo, kt, :],
                                praw[:, 0:512],
                                start=kt == 0, stop=kt == NKT - 1,
                                tile_position=(0, 0))
                        for pv, praw, jlo in ((pvA, prawA, 0), (pvB, prawB, 2)):
                            nc.tensor.matmul(
                                pv[64:97, :], vones[:, jlo + 1, kt, :],
                                praw[:, 512:1024],
                                start=False, stop=kt == NKT - 1,
                                tile_position=(0, 64),
                                skip_group_check=True)

                    prev = None
                    for kt in range(NKT):
                        ktp, kt2 = kt // 2, kt % 2
                        ksl = slice(kt * 128, (kt + 1) * 128)
                        sA = ps_s.tile([128, 1024], f32, tag="s",
                                       name=f"sA_{jq}_{kt}")
                        sB = ps_s.tile([128, 1024], f32, tag="s",
                                       name=f"sB_{jq}_{kt}")
                        # 4 packed score matmuls (row band = head)
                        for j in range(4):
                            s_t = sA if j < 2 else sB
                            half = slice((j % 2) * 512, (j % 2) * 512 + 512)
                            nc.tensor.matmul(
                                s_t[:, half],
                                kTb[32 * j:32 * (j + 1), ksl],
                                qTb[32 * j:32 * (j + 1), qsl],
                                start=True, stop=False,
                                tile_position=(32 * j, 0))
                        # bias accumulate via identity matmul
                        for j in range(4):
                            s_t = sA if j < 2 else sB
                            half = slice((j % 2) * 512, (j % 2) * 512 + 512)
                            bsl = slice(kt2 * 1024 + jq * 512,
                                        kt2 * 1024 + jq * 512 + 512)
                            nc.tensor.matmul(
                                s_t[:, half], ident, bp[j][ktp][:, bsl],
                                start=False, stop=True)
                        prawA = pp.tile([128, 1024], bf16, tag="praw",
                                        name=f"prawA_{jq}_{kt}")
                        nc.scalar.activation(prawA, sA, AF.Exp)
                        prawB = pp.tile([128, 1024], bf16, tag="praw",
                                        name=f"prawB_{jq}_{kt}")
                        nc.scalar.activation(prawB, sB, AF.Exp)
                        if prev is not None:
                            emit_pv(*prev)
                        prev = (kt, prawA, prawB)
                    emit_pv(*prev)

                    # gate + extract denominators, normalize, project out
                    # upper-half STT first: it waits on the bank's LAST
                    # matmul (h1/h3 kt=7), so the later lower-half read can't
                    # collide with an in-flight PE write to the same bank
                    for pv, jlo in ((pvA, 0), (pvB, 2)):
                        nc.vector.scalar_tensor_tensor(
                            odn[jlo + 1][0:33, qsl], pv[64:97, :], 1.0,
                            gT33[0:33, jlo + 1, qsl],
                            op0=ALU.mult, op1=ALU.mult)
                        nc.vector.scalar_tensor_tensor(
                            odn[jlo][0:33, qsl], pv[0:33, :], 1.0,
                            gT33[0:33, jlo, qsl],
                            op0=ALU.mult, op1=ALU.mult)
                    # denominators -> per-partition layout: den4[p, c] with
                    # c = 8h + 4jq + j4 covering q = 512 jq + 4p + j4
                    for j in range(4):
                        nc.scalar.dma_start(den4[:, 8 * j + 4 * jq:
                                                 8 * j + 4 * jq + 4],
                                            odn[j][32:33, qsl])
                    rsl = den4[:, :].rearrange("p (h t j) -> p h t j", h=4, t=2)
                    osl = recip4[:, :].rearrange("p (h t j) -> p h t j", h=4, t=2)
                    nc.vector.reciprocal(osl[:, :, jq, :], rsl[:, :, jq, :])
                    # per-head output projection + recip-scaled eviction
                    for j4 in range(4):
                        fos = []
                        for pi, jlo in ((0, 0), (1, 2)):
                            fo = ps_pv.tile([128, 512], f32, tag="pv",
                                            name=f"fo{pi}_{jq}_{j4}")
                            for ih in range(2):
                                j = jlo + ih
                                og = odn[j][0:C, qsl].rearrange(
                                    "c (p j) -> c j p", j=4)
                                nc.tensor.matmul(fo[:, ih * D:(ih + 1) * D],
                                                 og[:, j4, :], woh[:, j, :],
                                                 start=True, stop=True)
                            fos.append(fo)
                        # evict: reverse order so the first read waits on the
                        # bank's last matmul
                        acc = None
                        for j in (3, 2, 1, 0):
                            fo = fos[j // 2]
                            half = slice((j % 2) * D, (j % 2) * D + D)
                            rc = recip4[:, 8 * j + 4 * jq + j4:
                                        8 * j + 4 * jq + j4 + 1]
                            ot = outp.tile([128, D], f32, tag="ot",
                                           name=f"ot_{jq}_{j4}_{j}")
                            if acc is None:
                                nc.vector.tensor_scalar(ot, fo[:, half], rc,
                                                        None, op0=ALU.mult)
                            else:
                                nc.vector.scalar_tensor_tensor(
                                    ot, fo[:, half], rc, acc,
                                    op0=ALU.mult, op1=ALU.add)
                            acc = ot
                        nc.scalar.dma_start(out_r[jq, j4], acc)

    nc.finalize()
    return nc


def _get_nc():
    if "nc" not in _CACHE:
        _CACHE["nc"] = _build_nc()
    return _CACHE["nc"]


def _prep(inputs):
    q_x = np.asarray(inputs["q_x"], np.float32)
    kv_x = np.asarray(inputs["kv_x"], np.float32)
    bias_mask = np.asarray(inputs["bias_mask"], np.float32)
    bias_pair = np.asarray(inputs["bias_pair"], np.float32)
    wq = np.asarray(inputs["wq"], np.float32)
    wk = np.asarray(inputs["wk"], np.float32)
    wv = np.asarray(inputs["wv"], np.float32)
    wg = np.asarray(inputs["wg"], np.float32)
    bg = np.asarray(inputs["bg"], np.float32)
    wo = np.asarray(inputs["wo"], np.float32)

    sc = 1.0 / math.sqrt(C)
    ident = np.eye(128, dtype=_BF16)
    bmk = bias_mask.reshape(B, KL)

    in_maps = []
    for core in range(NCORES):
        b, hg = core // 2, core % 2
        hsl = slice(hg * 128, (hg + 1) * 128)
        qxT = np.ascontiguousarray(q_x[b].T).astype(_BF16).reshape(2, 128, Q)
        kvxT = np.ascontiguousarray(kv_x[b].T).astype(_BF16).reshape(2, 128, KL)
        # bias: [4h, K, Q] -> [4h, ktpair, p, kt2, q] -> [4, 4, 128, 2048]
        bT = (bias_pair[b, 4 * hg:4 * hg + 4].transpose(0, 2, 1)
              + bmk[b][None, :, None])
        bT = bT.reshape(4, 4, 2, 128, Q).transpose(0, 1, 3, 2, 4)
        bT = np.ascontiguousarray(bT).astype(_BF16).reshape(4, 4, 128, 2048)

        in_maps.append({
            "qxT": qxT,
            "kvxT": kvxT,
            "wq4": np.ascontiguousarray(wq[:, hsl] * sc).astype(_BF16).reshape(2, 128, 128),
            "wg4": np.ascontiguousarray(wg[:, hsl]).astype(_BF16).reshape(2, 128, 128),
            "wk4": np.ascontiguousarray(wk[:, hsl]).astype(_BF16).reshape(2, 128, 128),
            "wv4": np.ascontiguousarray(wv[:, hsl]).astype(_BF16).reshape(2, 128, 128),
            "bg4": (0.5 * bg[hsl]).astype(np.float32).reshape(128, 1),
            "biasT": bT,
            "ident": ident,
            "woh": np.ascontiguousarray(
                wo[hsl].reshape(4, C, D).transpose(1, 0, 2)
            ).astype(_BF16).reshape(C, 4 * D),
        })
    return in_maps


def _run(inputs, trace=False, **kw):
    from concourse.bass_utils import run_bass_kernel_spmd

    in_maps = _prep(inputs)
    nc = _get_nc()
    r = run_bass_kernel_spmd(nc, in_maps, core_ids=list(range(NCORES)),
                             trace=trace, **kw)
    bo = np.asarray(inputs["bo"], np.float32)
    out = np.zeros((B, Q, D), np.float32)
    for b in range(B):
        out[b] = (r.results[2 * b]["out"].astype(np.float32)
                  + r.results[2 * b + 1]["out"].astype(np.float32) + bo)
    return out, r


def kernel(**inputs):
    out, _ = _run(inputs, trace=False)
    return out
